# revision 2
# baseline (speedup 1.0000x reference)
"""nn_KernelCouplingA_CausalPool kernel: 8-way sharded (batch x head-group).

Distribution: core i handles batch b=i//4, heads [4g, 4g+4) with g=i%4.
Each core computes the projections/RoPE/gb for its batch (replicated across
the 4 cores of that batch) and the causal attention + attn-weight partial
for its own 4 heads. Host reassembles head outputs and sums the per-group
attention-probability partials into the mean over 16 heads.
"""
import math

import numpy as np

PHI = (1.0 + math.sqrt(5.0)) / 2.0
BASE_FAST = PHI
BASE_SLOW = 1618.0
D = 1024
NH = 16
HD = D // NH
B, L = 2, 2048
N_CORES = 8
GROUPS_PER_BATCH = 4
HEADS_PER_GROUP = NH // GROUPS_PER_BATCH


def _rope_tables(base):
    idx = np.arange(0, D, 2, dtype=np.float32) / float(D)
    inv = 1.0 / (base ** idx)
    t = np.arange(L, dtype=np.float32)
    freqs = np.outer(t, inv).astype(np.float32)
    emb = np.concatenate([freqs, freqs], axis=-1)
    return np.cos(emb).astype(np.float32), np.sin(emb).astype(np.float32)


def _rotate_half(x, xp):
    d = x.shape[-1]
    return xp.concatenate([-x[..., d // 2:], x[..., : d // 2]], axis=-1)


_PMAP_CACHE = {}


def _get_pmapped():
    if "fn" in _PMAP_CACHE:
        return _PMAP_CACHE["fn"]
    import jax
    import jax.numpy as jnp

    jax.config.update("jax_default_matmul_precision", "highest")
    devs = jax.devices()
    if len(devs) < N_CORES:
        raise RuntimeError(f"need {N_CORES} devices, have {len(devs)}")

    cos_f, sin_f = _rope_tables(BASE_FAST)
    cos_s, sin_s = _rope_tables(BASE_SLOW)
    causal = np.tril(np.ones((L, L), dtype=bool))

    def per_core(z_b, pool_b, head_start, Wq, bq, Wk, bk, Wv, bv, W_coh,
                 gamma, Wg1, bg1, Wg2, bg2):
        Q = z_b @ Wq.T + bq
        K_raw = z_b @ Wk.T + bk
        V = z_b @ Wv.T + bv

        gk = pool_b @ W_coh.T
        gb = jnp.tanh(gamma[0]) * (gk @ K_raw.T) * (D ** -0.5)

        h = jax.nn.gelu(pool_b @ Wg1.T + bg1, approximate=False)
        gate = jax.nn.sigmoid(h @ Wg2.T + bg2)

        cos = gate * cos_f + (1.0 - gate) * cos_s
        sin = gate * sin_f + (1.0 - gate) * sin_s
        Qr = Q * cos + _rotate_half(Q, jnp) * sin
        Kr = K_raw * cos + _rotate_half(K_raw, jnp) * sin

        Qh = Qr.reshape(L, NH, HD)
        Kh = Kr.reshape(L, NH, HD)
        Vh = V.reshape(L, NH, HD)
        Qh = jax.lax.dynamic_slice_in_dim(Qh, head_start, HEADS_PER_GROUP, axis=1)
        Kh = jax.lax.dynamic_slice_in_dim(Kh, head_start, HEADS_PER_GROUP, axis=1)
        Vh = jax.lax.dynamic_slice_in_dim(Vh, head_start, HEADS_PER_GROUP, axis=1)

        logits = jnp.einsum("qhd,khd->hqk", Qh, Kh) * (HD ** -0.5)
        logits = logits + gb[None, :, :]
        logits = jnp.where(causal[None], logits, jnp.float32(-1e30))
        attn = jax.nn.softmax(logits, axis=-1)
        out_part = jnp.einsum("hqk,khd->qhd", attn, Vh)
        attn_part = attn.sum(axis=0)
        return out_part, attn_part

    _PMAP_CACHE["fn"] = jax.pmap(
        per_core, in_axes=(0, 0, 0) + (None,) * 14)
    return _PMAP_CACHE["fn"]


def _kernel_jax(z, pool_seq, Wq, bq, Wk, bk, Wv, bv, W_coh, gamma,
                Wg1, bg1, Wg2, bg2):
    pmapped = _get_pmapped()
    z_dev = np.stack([z[i // GROUPS_PER_BATCH] for i in range(N_CORES)])
    pool_dev = np.stack([pool_seq[i // GROUPS_PER_BATCH] for i in range(N_CORES)])
    hstart = np.array([(i % GROUPS_PER_BATCH) * HEADS_PER_GROUP for i in range(N_CORES)],
                      dtype=np.int32)

    out_parts, attn_parts = pmapped(z_dev, pool_dev, hstart, Wq, bq, Wk, bk,
                                    Wv, bv, W_coh, gamma, Wg1, bg1, Wg2, bg2)
    out_parts = np.asarray(out_parts)
    attn_parts = np.asarray(attn_parts)

    out = np.empty((B, L, NH, HD), dtype=np.float32)
    attn_w = np.zeros((B, L, L), dtype=np.float32)
    for i in range(N_CORES):
        b = i // GROUPS_PER_BATCH
        g = i % GROUPS_PER_BATCH
        out[b, :, g * HEADS_PER_GROUP:(g + 1) * HEADS_PER_GROUP, :] = out_parts[i]
        attn_w[b] += attn_parts[i]
    attn_w /= NH
    return out.reshape(B, L, D), attn_w


def _kernel_numpy(z, pool_seq, Wq, bq, Wk, bk, Wv, bv, W_coh, gamma,
                  Wg1, bg1, Wg2, bg2):
    from scipy.special import erf

    cos_f, sin_f = _rope_tables(BASE_FAST)
    cos_s, sin_s = _rope_tables(BASE_SLOW)

    Q = z @ Wq.T + bq
    K_raw = z @ Wk.T + bk
    V = z @ Wv.T + bv

    gk = pool_seq @ W_coh.T
    gb = np.tanh(gamma[0]) * np.einsum("bqd,bkd->bqk", gk, K_raw) * (D ** -0.5)

    h_pre = pool_seq @ Wg1.T + bg1
    h = 0.5 * h_pre * (1.0 + erf(h_pre / np.sqrt(2.0, dtype=np.float32)))
    gate = 1.0 / (1.0 + np.exp(-(h @ Wg2.T + bg2)))

    cos = gate * cos_f + (1.0 - gate) * cos_s
    sin = gate * sin_f + (1.0 - gate) * sin_s
    Q = Q * cos + _rotate_half(Q, np) * sin
    K_rot = K_raw * cos + _rotate_half(K_raw, np) * sin

    Qh = Q.reshape(B, L, NH, HD).transpose(0, 2, 1, 3)
    Kh = K_rot.reshape(B, L, NH, HD).transpose(0, 2, 1, 3)
    Vh = V.reshape(B, L, NH, HD).transpose(0, 2, 1, 3)

    logits = np.einsum("bhqd,bhkd->bhqk", Qh, Kh).astype(np.float32) * (HD ** -0.5)
    logits = logits + gb[:, None]
    causal = np.tril(np.ones((L, L), dtype=bool))
    logits = np.where(causal[None, None], logits, np.float32(-np.inf))

    logits -= logits.max(axis=-1, keepdims=True)
    ex = np.exp(logits, dtype=np.float32)
    attn = ex / ex.sum(axis=-1, keepdims=True)
    out = np.einsum("bhqk,bhkd->bhqd", attn, Vh).astype(np.float32)
    out = out.transpose(0, 2, 1, 3).reshape(B, L, D)
    return out.astype(np.float32), attn.mean(axis=1).astype(np.float32)


def kernel(**inputs):
    inputs = {k: np.asarray(v, dtype=np.float32) for k, v in inputs.items()}
    try:
        return _kernel_jax(**inputs)
    except Exception:
        return _kernel_numpy(**inputs)


# revision 3
# speedup vs baseline: 1.1986x; 1.1986x over previous
"""nn_KernelCouplingA_CausalPool kernel: 8-way sharded (batch x head-group).

Distribution: core i handles batch b=i//4, heads [4g, 4g+4) with g=i%4.
Each core computes the projections/RoPE/gb for its batch (replicated across
the 4 cores of that batch) and the causal attention + attn-weight partial
for its own 4 heads. Host reassembles head outputs and sums the per-group
attention-probability partials into the mean over 16 heads.
"""
import math

import numpy as np

PHI = (1.0 + math.sqrt(5.0)) / 2.0
BASE_FAST = PHI
BASE_SLOW = 1618.0
D = 1024
NH = 16
HD = D // NH
B, L = 2, 2048
N_CORES = 8
GROUPS_PER_BATCH = 4
HEADS_PER_GROUP = NH // GROUPS_PER_BATCH


def _rope_tables(base):
    idx = np.arange(0, D, 2, dtype=np.float32) / float(D)
    inv = 1.0 / (base ** idx)
    t = np.arange(L, dtype=np.float32)
    freqs = np.outer(t, inv).astype(np.float32)
    emb = np.concatenate([freqs, freqs], axis=-1)
    return np.cos(emb).astype(np.float32), np.sin(emb).astype(np.float32)


def _rotate_half(x, xp):
    d = x.shape[-1]
    return xp.concatenate([-x[..., d // 2:], x[..., : d // 2]], axis=-1)


_PMAP_CACHE = {}


def _get_pmapped():
    if "fn" in _PMAP_CACHE:
        return _PMAP_CACHE["fn"]
    import jax
    import jax.numpy as jnp

    jax.config.update("jax_default_matmul_precision", "highest")
    devs = jax.devices()
    if len(devs) < N_CORES:
        raise RuntimeError(f"need {N_CORES} devices, have {len(devs)}")

    cos_f, sin_f = _rope_tables(BASE_FAST)
    cos_s, sin_s = _rope_tables(BASE_SLOW)
    causal = np.tril(np.ones((L, L), dtype=bool))

    def per_core(z_b, pool_b, head_start, Wq, bq, Wk, bk, Wv, bv, W_coh,
                 gamma, Wg1, bg1, Wg2, bg2):
        Q = z_b @ Wq.T + bq
        K_raw = z_b @ Wk.T + bk
        V = z_b @ Wv.T + bv

        gk = pool_b @ W_coh.T
        gb = jnp.tanh(gamma[0]) * (gk @ K_raw.T) * (D ** -0.5)

        h = jax.nn.gelu(pool_b @ Wg1.T + bg1, approximate=False)
        gate = jax.nn.sigmoid(h @ Wg2.T + bg2)

        cos = gate * cos_f + (1.0 - gate) * cos_s
        sin = gate * sin_f + (1.0 - gate) * sin_s
        Qr = Q * cos + _rotate_half(Q, jnp) * sin
        Kr = K_raw * cos + _rotate_half(K_raw, jnp) * sin

        Qh = Qr.reshape(L, NH, HD)
        Kh = Kr.reshape(L, NH, HD)
        Vh = V.reshape(L, NH, HD)
        Qh = jax.lax.dynamic_slice_in_dim(Qh, head_start, HEADS_PER_GROUP, axis=1)
        Kh = jax.lax.dynamic_slice_in_dim(Kh, head_start, HEADS_PER_GROUP, axis=1)
        Vh = jax.lax.dynamic_slice_in_dim(Vh, head_start, HEADS_PER_GROUP, axis=1)

        logits = jnp.einsum("qhd,khd->hqk", Qh, Kh) * (HD ** -0.5)
        logits = logits + gb[None, :, :]
        logits = jnp.where(causal[None], logits, jnp.float32(-1e30))
        attn = jax.nn.softmax(logits, axis=-1)
        out_part = jnp.einsum("hqk,khd->qhd", attn, Vh)
        attn_part = attn.sum(axis=0)
        return out_part, attn_part

    _PMAP_CACHE["fn"] = jax.pmap(
        per_core, in_axes=(0, 0, 0) + (None,) * 14)
    return _PMAP_CACHE["fn"]



_PMAP_Q_CACHE = {}
Q_PER_CORE = L // GROUPS_PER_BATCH


def _get_pmapped_q():
    if "fn" in _PMAP_Q_CACHE:
        return _PMAP_Q_CACHE["fn"]
    import jax
    import jax.numpy as jnp
    from jax import lax

    jax.config.update("jax_default_matmul_precision", "highest")
    if len(jax.devices()) < N_CORES:
        raise RuntimeError("need 8 devices")

    cos_f, sin_f = _rope_tables(BASE_FAST)
    cos_s, sin_s = _rope_tables(BASE_SLOW)
    groups = [[0, 1, 2, 3], [4, 5, 6, 7]]

    def per_core(z_q, pool_q, qstart, Wq, bq, Wk, bk, Wv, bv, W_coh,
                 gamma, Wg1, bg1, Wg2, bg2):
        Q_q = z_q @ Wq.T + bq
        K_q = z_q @ Wk.T + bk
        V_q = z_q @ Wv.T + bv
        gk_q = pool_q @ W_coh.T
        h_q = jax.nn.gelu(pool_q @ Wg1.T + bg1, approximate=False)
        gate_q = jax.nn.sigmoid(h_q @ Wg2.T + bg2)

        K_raw = lax.all_gather(K_q, "i", axis_index_groups=groups).reshape(L, D)
        V = lax.all_gather(V_q, "i", axis_index_groups=groups).reshape(L, D)
        gate = lax.all_gather(gate_q, "i", axis_index_groups=groups).reshape(L, 1)

        cos = gate * cos_f + (1.0 - gate) * cos_s
        sin = gate * sin_f + (1.0 - gate) * sin_s
        Kr = K_raw * cos + _rotate_half(K_raw, jnp) * sin
        cos_q = lax.dynamic_slice_in_dim(cos, qstart, Q_PER_CORE, axis=0)
        sin_q = lax.dynamic_slice_in_dim(sin, qstart, Q_PER_CORE, axis=0)
        Qr_q = Q_q * cos_q + _rotate_half(Q_q, jnp) * sin_q

        gb_q = jnp.tanh(gamma[0]) * (gk_q @ K_raw.T) * (D ** -0.5)

        Qh = Qr_q.reshape(Q_PER_CORE, NH, HD)
        Kh = Kr.reshape(L, NH, HD)
        Vh = V.reshape(L, NH, HD)
        logits = jnp.einsum("qhd,khd->hqk", Qh, Kh) * (HD ** -0.5)
        logits = logits + gb_q[None]
        kcol = jnp.arange(L, dtype=jnp.int32)[None, :]
        qrow = qstart + jnp.arange(Q_PER_CORE, dtype=jnp.int32)[:, None]
        logits = jnp.where((kcol <= qrow)[None], logits, jnp.float32(-1e30))
        attn = jax.nn.softmax(logits, axis=-1)
        out_q = jnp.einsum("hqk,khd->qhd", attn, Vh).reshape(Q_PER_CORE, D)
        attn_w_q = attn.mean(axis=0)
        return out_q, attn_w_q

    _PMAP_Q_CACHE["fn"] = jax.pmap(
        per_core, axis_name="i", in_axes=(0, 0, 0) + (None,) * 14)
    return _PMAP_Q_CACHE["fn"]


def _kernel_jax_q(z, pool_seq, Wq, bq, Wk, bk, Wv, bv, W_coh, gamma,
                  Wg1, bg1, Wg2, bg2):
    pmapped = _get_pmapped_q()
    z_dev = np.stack([
        z[i // GROUPS_PER_BATCH,
          (i % GROUPS_PER_BATCH) * Q_PER_CORE:(i % GROUPS_PER_BATCH + 1) * Q_PER_CORE]
        for i in range(N_CORES)])
    pool_dev = np.stack([
        pool_seq[i // GROUPS_PER_BATCH,
                 (i % GROUPS_PER_BATCH) * Q_PER_CORE:(i % GROUPS_PER_BATCH + 1) * Q_PER_CORE]
        for i in range(N_CORES)])
    qstart = np.array([(i % GROUPS_PER_BATCH) * Q_PER_CORE
                       for i in range(N_CORES)], dtype=np.int32)

    out_parts, attn_parts = pmapped(z_dev, pool_dev, qstart, Wq, bq, Wk, bk,
                                    Wv, bv, W_coh, gamma, Wg1, bg1, Wg2, bg2)
    out_parts = np.asarray(out_parts)
    attn_parts = np.asarray(attn_parts)

    out = np.empty((B, L, D), dtype=np.float32)
    attn_w = np.empty((B, L, L), dtype=np.float32)
    for i in range(N_CORES):
        b = i // GROUPS_PER_BATCH
        g = i % GROUPS_PER_BATCH
        out[b, g * Q_PER_CORE:(g + 1) * Q_PER_CORE] = out_parts[i]
        attn_w[b, g * Q_PER_CORE:(g + 1) * Q_PER_CORE] = attn_parts[i]
    return out, attn_w


def _kernel_jax(z, pool_seq, Wq, bq, Wk, bk, Wv, bv, W_coh, gamma,
                Wg1, bg1, Wg2, bg2):
    pmapped = _get_pmapped()
    z_dev = np.stack([z[i // GROUPS_PER_BATCH] for i in range(N_CORES)])
    pool_dev = np.stack([pool_seq[i // GROUPS_PER_BATCH] for i in range(N_CORES)])
    hstart = np.array([(i % GROUPS_PER_BATCH) * HEADS_PER_GROUP for i in range(N_CORES)],
                      dtype=np.int32)

    out_parts, attn_parts = pmapped(z_dev, pool_dev, hstart, Wq, bq, Wk, bk,
                                    Wv, bv, W_coh, gamma, Wg1, bg1, Wg2, bg2)
    out_parts = np.asarray(out_parts)
    attn_parts = np.asarray(attn_parts)

    out = np.empty((B, L, NH, HD), dtype=np.float32)
    attn_w = np.zeros((B, L, L), dtype=np.float32)
    for i in range(N_CORES):
        b = i // GROUPS_PER_BATCH
        g = i % GROUPS_PER_BATCH
        out[b, :, g * HEADS_PER_GROUP:(g + 1) * HEADS_PER_GROUP, :] = out_parts[i]
        attn_w[b] += attn_parts[i]
    attn_w /= NH
    return out.reshape(B, L, D), attn_w


def _kernel_numpy(z, pool_seq, Wq, bq, Wk, bk, Wv, bv, W_coh, gamma,
                  Wg1, bg1, Wg2, bg2):
    from scipy.special import erf

    cos_f, sin_f = _rope_tables(BASE_FAST)
    cos_s, sin_s = _rope_tables(BASE_SLOW)

    Q = z @ Wq.T + bq
    K_raw = z @ Wk.T + bk
    V = z @ Wv.T + bv

    gk = pool_seq @ W_coh.T
    gb = np.tanh(gamma[0]) * np.einsum("bqd,bkd->bqk", gk, K_raw) * (D ** -0.5)

    h_pre = pool_seq @ Wg1.T + bg1
    h = 0.5 * h_pre * (1.0 + erf(h_pre / np.sqrt(2.0, dtype=np.float32)))
    gate = 1.0 / (1.0 + np.exp(-(h @ Wg2.T + bg2)))

    cos = gate * cos_f + (1.0 - gate) * cos_s
    sin = gate * sin_f + (1.0 - gate) * sin_s
    Q = Q * cos + _rotate_half(Q, np) * sin
    K_rot = K_raw * cos + _rotate_half(K_raw, np) * sin

    Qh = Q.reshape(B, L, NH, HD).transpose(0, 2, 1, 3)
    Kh = K_rot.reshape(B, L, NH, HD).transpose(0, 2, 1, 3)
    Vh = V.reshape(B, L, NH, HD).transpose(0, 2, 1, 3)

    logits = np.einsum("bhqd,bhkd->bhqk", Qh, Kh).astype(np.float32) * (HD ** -0.5)
    logits = logits + gb[:, None]
    causal = np.tril(np.ones((L, L), dtype=bool))
    logits = np.where(causal[None, None], logits, np.float32(-np.inf))

    logits -= logits.max(axis=-1, keepdims=True)
    ex = np.exp(logits, dtype=np.float32)
    attn = ex / ex.sum(axis=-1, keepdims=True)
    out = np.einsum("bhqk,bhkd->bhqd", attn, Vh).astype(np.float32)
    out = out.transpose(0, 2, 1, 3).reshape(B, L, D)
    return out.astype(np.float32), attn.mean(axis=1).astype(np.float32)


def kernel(**inputs):
    inputs = {k: np.asarray(v, dtype=np.float32) for k, v in inputs.items()}
    try:
        return _kernel_jax_q(**inputs)
    except Exception:
        pass
    try:
        return _kernel_jax(**inputs)
    except Exception:
        return _kernel_numpy(**inputs)


# revision 4
# speedup vs baseline: 5.2351x; 4.3678x over previous
"""nn_KernelCouplingA_CausalPool kernel: 8-way sharded (batch x head-group).

Distribution: core i handles batch b=i//4, heads [4g, 4g+4) with g=i%4.
Each core computes the projections/RoPE/gb for its batch (replicated across
the 4 cores of that batch) and the causal attention + attn-weight partial
for its own 4 heads. Host reassembles head outputs and sums the per-group
attention-probability partials into the mean over 16 heads.
"""
import math

import numpy as np

PHI = (1.0 + math.sqrt(5.0)) / 2.0
BASE_FAST = PHI
BASE_SLOW = 1618.0
D = 1024
NH = 16
HD = D // NH
B, L = 2, 2048
N_CORES = 8
GROUPS_PER_BATCH = 4
HEADS_PER_GROUP = NH // GROUPS_PER_BATCH


def _rope_tables(base):
    idx = np.arange(0, D, 2, dtype=np.float32) / float(D)
    inv = 1.0 / (base ** idx)
    t = np.arange(L, dtype=np.float32)
    freqs = np.outer(t, inv).astype(np.float32)
    emb = np.concatenate([freqs, freqs], axis=-1)
    return np.cos(emb).astype(np.float32), np.sin(emb).astype(np.float32)


def _rotate_half(x, xp):
    d = x.shape[-1]
    return xp.concatenate([-x[..., d // 2:], x[..., : d // 2]], axis=-1)


_PMAP_CACHE = {}


def _get_pmapped():
    if "fn" in _PMAP_CACHE:
        return _PMAP_CACHE["fn"]
    import jax
    import jax.numpy as jnp

    jax.config.update("jax_default_matmul_precision", "highest")
    devs = jax.devices()
    if len(devs) < N_CORES:
        raise RuntimeError(f"need {N_CORES} devices, have {len(devs)}")

    cos_f, sin_f = _rope_tables(BASE_FAST)
    cos_s, sin_s = _rope_tables(BASE_SLOW)
    causal = np.tril(np.ones((L, L), dtype=bool))

    def per_core(z_b, pool_b, head_start, Wq, bq, Wk, bk, Wv, bv, W_coh,
                 gamma, Wg1, bg1, Wg2, bg2):
        Q = z_b @ Wq.T + bq
        K_raw = z_b @ Wk.T + bk
        V = z_b @ Wv.T + bv

        gk = pool_b @ W_coh.T
        gb = jnp.tanh(gamma[0]) * (gk @ K_raw.T) * (D ** -0.5)

        h = jax.nn.gelu(pool_b @ Wg1.T + bg1, approximate=False)
        gate = jax.nn.sigmoid(h @ Wg2.T + bg2)

        cos = gate * cos_f + (1.0 - gate) * cos_s
        sin = gate * sin_f + (1.0 - gate) * sin_s
        Qr = Q * cos + _rotate_half(Q, jnp) * sin
        Kr = K_raw * cos + _rotate_half(K_raw, jnp) * sin

        Qh = Qr.reshape(L, NH, HD)
        Kh = Kr.reshape(L, NH, HD)
        Vh = V.reshape(L, NH, HD)
        Qh = jax.lax.dynamic_slice_in_dim(Qh, head_start, HEADS_PER_GROUP, axis=1)
        Kh = jax.lax.dynamic_slice_in_dim(Kh, head_start, HEADS_PER_GROUP, axis=1)
        Vh = jax.lax.dynamic_slice_in_dim(Vh, head_start, HEADS_PER_GROUP, axis=1)

        logits = jnp.einsum("qhd,khd->hqk", Qh, Kh) * (HD ** -0.5)
        logits = logits + gb[None, :, :]
        logits = jnp.where(causal[None], logits, jnp.float32(-1e30))
        attn = jax.nn.softmax(logits, axis=-1)
        out_part = jnp.einsum("hqk,khd->qhd", attn, Vh)
        attn_part = attn.sum(axis=0)
        return out_part, attn_part

    _PMAP_CACHE["fn"] = jax.pmap(
        per_core, in_axes=(0, 0, 0) + (None,) * 12)
    return _PMAP_CACHE["fn"]



_PMAP_Q_CACHE = {}
Q_PER_CORE = L // GROUPS_PER_BATCH


def _get_pmapped_q():
    if "fn" in _PMAP_Q_CACHE:
        return _PMAP_Q_CACHE["fn"]
    import jax
    import jax.numpy as jnp
    from jax import lax

    jax.config.update("jax_default_matmul_precision", "highest")
    if len(jax.devices()) < N_CORES:
        raise RuntimeError("need 8 devices")

    cos_f, sin_f = _rope_tables(BASE_FAST)
    cos_s, sin_s = _rope_tables(BASE_SLOW)
    groups = [[0, 1, 2, 3], [4, 5, 6, 7]]

    def per_core(z_q, pool_q, qstart, Wq, bq, Wk, bk, Wv, bv, W_coh,
                 gamma, Wg1, bg1, Wg2, bg2):
        Q_q = z_q @ Wq.T + bq
        K_q = z_q @ Wk.T + bk
        V_q = z_q @ Wv.T + bv
        gk_q = pool_q @ W_coh.T
        h_q = jax.nn.gelu(pool_q @ Wg1.T + bg1, approximate=False)
        gate_q = jax.nn.sigmoid(h_q @ Wg2.T + bg2)

        K_raw = lax.all_gather(K_q, "i", axis_index_groups=groups).reshape(L, D)
        V = lax.all_gather(V_q, "i", axis_index_groups=groups).reshape(L, D)
        gate = lax.all_gather(gate_q, "i", axis_index_groups=groups).reshape(L, 1)

        cos = gate * cos_f + (1.0 - gate) * cos_s
        sin = gate * sin_f + (1.0 - gate) * sin_s
        Kr = K_raw * cos + _rotate_half(K_raw, jnp) * sin
        cos_q = lax.dynamic_slice_in_dim(cos, qstart, Q_PER_CORE, axis=0)
        sin_q = lax.dynamic_slice_in_dim(sin, qstart, Q_PER_CORE, axis=0)
        Qr_q = Q_q * cos_q + _rotate_half(Q_q, jnp) * sin_q

        gb_q = jnp.tanh(gamma[0]) * (gk_q @ K_raw.T) * (D ** -0.5)

        Qh = Qr_q.reshape(Q_PER_CORE, NH, HD)
        Kh = Kr.reshape(L, NH, HD)
        Vh = V.reshape(L, NH, HD)
        logits = jnp.einsum("qhd,khd->hqk", Qh, Kh) * (HD ** -0.5)
        logits = logits + gb_q[None]
        kcol = jnp.arange(L, dtype=jnp.int32)[None, :]
        qrow = qstart + jnp.arange(Q_PER_CORE, dtype=jnp.int32)[:, None]
        logits = jnp.where((kcol <= qrow)[None], logits, jnp.float32(-1e30))
        attn = jax.nn.softmax(logits, axis=-1)
        out_q = jnp.einsum("hqk,khd->qhd", attn, Vh).reshape(Q_PER_CORE, D)
        attn_w_q = attn.mean(axis=0)
        return out_q, attn_w_q

    _PMAP_Q_CACHE["fn"] = jax.pmap(
        per_core, axis_name="i", in_axes=(0, 0, 0) + (None,) * 12)
    return _PMAP_Q_CACHE["fn"]


def _kernel_jax_q(z, pool_seq, Wq, bq, Wk, bk, Wv, bv, W_coh, gamma,
                  Wg1, bg1, Wg2, bg2):
    pmapped = _get_pmapped_q()
    z_dev = np.stack([
        z[i // GROUPS_PER_BATCH,
          (i % GROUPS_PER_BATCH) * Q_PER_CORE:(i % GROUPS_PER_BATCH + 1) * Q_PER_CORE]
        for i in range(N_CORES)])
    pool_dev = np.stack([
        pool_seq[i // GROUPS_PER_BATCH,
                 (i % GROUPS_PER_BATCH) * Q_PER_CORE:(i % GROUPS_PER_BATCH + 1) * Q_PER_CORE]
        for i in range(N_CORES)])
    qstart = np.array([(i % GROUPS_PER_BATCH) * Q_PER_CORE
                       for i in range(N_CORES)], dtype=np.int32)

    out_parts, attn_parts = pmapped(z_dev, pool_dev, qstart, Wq, bq, Wk, bk,
                                    Wv, bv, W_coh, gamma, Wg1, bg1, Wg2, bg2)
    out_parts = np.asarray(out_parts)
    attn_parts = np.asarray(attn_parts)

    out = np.empty((B, L, D), dtype=np.float32)
    attn_w = np.empty((B, L, L), dtype=np.float32)
    for i in range(N_CORES):
        b = i // GROUPS_PER_BATCH
        g = i % GROUPS_PER_BATCH
        out[b, g * Q_PER_CORE:(g + 1) * Q_PER_CORE] = out_parts[i]
        attn_w[b, g * Q_PER_CORE:(g + 1) * Q_PER_CORE] = attn_parts[i]
    return out, attn_w


def _kernel_jax(z, pool_seq, Wq, bq, Wk, bk, Wv, bv, W_coh, gamma,
                Wg1, bg1, Wg2, bg2):
    pmapped = _get_pmapped()
    z_dev = np.stack([z[i // GROUPS_PER_BATCH] for i in range(N_CORES)])
    pool_dev = np.stack([pool_seq[i // GROUPS_PER_BATCH] for i in range(N_CORES)])
    hstart = np.array([(i % GROUPS_PER_BATCH) * HEADS_PER_GROUP for i in range(N_CORES)],
                      dtype=np.int32)

    out_parts, attn_parts = pmapped(z_dev, pool_dev, hstart, Wq, bq, Wk, bk,
                                    Wv, bv, W_coh, gamma, Wg1, bg1, Wg2, bg2)
    out_parts = np.asarray(out_parts)
    attn_parts = np.asarray(attn_parts)

    out = np.empty((B, L, NH, HD), dtype=np.float32)
    attn_w = np.zeros((B, L, L), dtype=np.float32)
    for i in range(N_CORES):
        b = i // GROUPS_PER_BATCH
        g = i % GROUPS_PER_BATCH
        out[b, :, g * HEADS_PER_GROUP:(g + 1) * HEADS_PER_GROUP, :] = out_parts[i]
        attn_w[b] += attn_parts[i]
    attn_w /= NH
    return out.reshape(B, L, D), attn_w


def _kernel_numpy(z, pool_seq, Wq, bq, Wk, bk, Wv, bv, W_coh, gamma,
                  Wg1, bg1, Wg2, bg2):
    from scipy.special import erf

    cos_f, sin_f = _rope_tables(BASE_FAST)
    cos_s, sin_s = _rope_tables(BASE_SLOW)

    Q = z @ Wq.T + bq
    K_raw = z @ Wk.T + bk
    V = z @ Wv.T + bv

    gk = pool_seq @ W_coh.T
    gb = np.tanh(gamma[0]) * np.einsum("bqd,bkd->bqk", gk, K_raw) * (D ** -0.5)

    h_pre = pool_seq @ Wg1.T + bg1
    h = 0.5 * h_pre * (1.0 + erf(h_pre / np.sqrt(2.0, dtype=np.float32)))
    gate = 1.0 / (1.0 + np.exp(-(h @ Wg2.T + bg2)))

    cos = gate * cos_f + (1.0 - gate) * cos_s
    sin = gate * sin_f + (1.0 - gate) * sin_s
    Q = Q * cos + _rotate_half(Q, np) * sin
    K_rot = K_raw * cos + _rotate_half(K_raw, np) * sin

    Qh = Q.reshape(B, L, NH, HD).transpose(0, 2, 1, 3)
    Kh = K_rot.reshape(B, L, NH, HD).transpose(0, 2, 1, 3)
    Vh = V.reshape(B, L, NH, HD).transpose(0, 2, 1, 3)

    logits = np.einsum("bhqd,bhkd->bhqk", Qh, Kh).astype(np.float32) * (HD ** -0.5)
    logits = logits + gb[:, None]
    causal = np.tril(np.ones((L, L), dtype=bool))
    logits = np.where(causal[None, None], logits, np.float32(-np.inf))

    logits -= logits.max(axis=-1, keepdims=True)
    ex = np.exp(logits, dtype=np.float32)
    attn = ex / ex.sum(axis=-1, keepdims=True)
    out = np.einsum("bhqk,bhkd->bhqd", attn, Vh).astype(np.float32)
    out = out.transpose(0, 2, 1, 3).reshape(B, L, D)
    return out.astype(np.float32), attn.mean(axis=1).astype(np.float32)


def kernel(**inputs):
    inputs = {k: np.asarray(v, dtype=np.float32) for k, v in inputs.items()}
    try:
        return _kernel_jax_q(**inputs)
    except Exception:
        pass
    try:
        return _kernel_jax(**inputs)
    except Exception:
        return _kernel_numpy(**inputs)


# revision 5
# speedup vs baseline: 5.9192x; 1.1307x over previous
"""nn_KernelCouplingA_CausalPool kernel: 8-way sharded (batch x head-group).

Distribution: core i handles batch b=i//4, heads [4g, 4g+4) with g=i%4.
Each core computes the projections/RoPE/gb for its batch (replicated across
the 4 cores of that batch) and the causal attention + attn-weight partial
for its own 4 heads. Host reassembles head outputs and sums the per-group
attention-probability partials into the mean over 16 heads.
"""
import math

import numpy as np

PHI = (1.0 + math.sqrt(5.0)) / 2.0
BASE_FAST = PHI
BASE_SLOW = 1618.0
D = 1024
NH = 16
HD = D // NH
B, L = 2, 2048
N_CORES = 8
GROUPS_PER_BATCH = 4
HEADS_PER_GROUP = NH // GROUPS_PER_BATCH


def _rope_tables(base):
    idx = np.arange(0, D, 2, dtype=np.float32) / float(D)
    inv = 1.0 / (base ** idx)
    t = np.arange(L, dtype=np.float32)
    freqs = np.outer(t, inv).astype(np.float32)
    emb = np.concatenate([freqs, freqs], axis=-1)
    return np.cos(emb).astype(np.float32), np.sin(emb).astype(np.float32)


def _rotate_half(x, xp):
    d = x.shape[-1]
    return xp.concatenate([-x[..., d // 2:], x[..., : d // 2]], axis=-1)


_PMAP_CACHE = {}


def _get_pmapped():
    if "fn" in _PMAP_CACHE:
        return _PMAP_CACHE["fn"]
    import jax
    import jax.numpy as jnp

    jax.config.update("jax_default_matmul_precision", "high")
    devs = jax.devices()
    if len(devs) < N_CORES:
        raise RuntimeError(f"need {N_CORES} devices, have {len(devs)}")

    cos_f, sin_f = _rope_tables(BASE_FAST)
    cos_s, sin_s = _rope_tables(BASE_SLOW)
    causal = np.tril(np.ones((L, L), dtype=bool))

    def per_core(z_b, pool_b, head_start, Wq, bq, Wk, bk, Wv, bv, W_coh,
                 gamma, Wg1, bg1, Wg2, bg2):
        Q = z_b @ Wq.T + bq
        K_raw = z_b @ Wk.T + bk
        V = z_b @ Wv.T + bv

        gk = pool_b @ W_coh.T
        gb = jnp.tanh(gamma[0]) * (gk @ K_raw.T) * (D ** -0.5)

        h = jax.nn.gelu(pool_b @ Wg1.T + bg1, approximate=False)
        gate = jax.nn.sigmoid(h @ Wg2.T + bg2)

        cos = gate * cos_f + (1.0 - gate) * cos_s
        sin = gate * sin_f + (1.0 - gate) * sin_s
        Qr = Q * cos + _rotate_half(Q, jnp) * sin
        Kr = K_raw * cos + _rotate_half(K_raw, jnp) * sin

        Qh = Qr.reshape(L, NH, HD)
        Kh = Kr.reshape(L, NH, HD)
        Vh = V.reshape(L, NH, HD)
        Qh = jax.lax.dynamic_slice_in_dim(Qh, head_start, HEADS_PER_GROUP, axis=1)
        Kh = jax.lax.dynamic_slice_in_dim(Kh, head_start, HEADS_PER_GROUP, axis=1)
        Vh = jax.lax.dynamic_slice_in_dim(Vh, head_start, HEADS_PER_GROUP, axis=1)

        logits = jnp.einsum("qhd,khd->hqk", Qh, Kh) * (HD ** -0.5)
        logits = logits + gb[None, :, :]
        logits = jnp.where(causal[None], logits, jnp.float32(-1e30))
        attn = jax.nn.softmax(logits, axis=-1)
        out_part = jnp.einsum("hqk,khd->qhd", attn, Vh)
        attn_part = attn.sum(axis=0)
        return out_part, attn_part

    _PMAP_CACHE["fn"] = jax.pmap(
        per_core, in_axes=(0, 0, 0) + (None,) * 12)
    return _PMAP_CACHE["fn"]



_PMAP_Q_CACHE = {}
Q_PER_CORE = L // GROUPS_PER_BATCH


def _get_pmapped_q():
    if "fn" in _PMAP_Q_CACHE:
        return _PMAP_Q_CACHE["fn"]
    import jax
    import jax.numpy as jnp
    from jax import lax

    jax.config.update("jax_default_matmul_precision", "high")
    if len(jax.devices()) < N_CORES:
        raise RuntimeError("need 8 devices")

    cos_f, sin_f = _rope_tables(BASE_FAST)
    cos_s, sin_s = _rope_tables(BASE_SLOW)
    groups = [[0, 1, 2, 3], [4, 5, 6, 7]]

    def per_core(z_q, pool_q, qstart, Wq, bq, Wk, bk, Wv, bv, W_coh,
                 gamma, Wg1, bg1, Wg2, bg2):
        Q_q = z_q @ Wq.T + bq
        K_q = z_q @ Wk.T + bk
        V_q = z_q @ Wv.T + bv
        gk_q = pool_q @ W_coh.T
        h_q = jax.nn.gelu(pool_q @ Wg1.T + bg1, approximate=False)
        gate_q = jax.nn.sigmoid(h_q @ Wg2.T + bg2)

        K_raw = lax.all_gather(K_q, "i", axis_index_groups=groups).reshape(L, D)
        V = lax.all_gather(V_q, "i", axis_index_groups=groups).reshape(L, D)
        gate = lax.all_gather(gate_q, "i", axis_index_groups=groups).reshape(L, 1)

        cos = gate * cos_f + (1.0 - gate) * cos_s
        sin = gate * sin_f + (1.0 - gate) * sin_s
        Kr = K_raw * cos + _rotate_half(K_raw, jnp) * sin
        cos_q = lax.dynamic_slice_in_dim(cos, qstart, Q_PER_CORE, axis=0)
        sin_q = lax.dynamic_slice_in_dim(sin, qstart, Q_PER_CORE, axis=0)
        Qr_q = Q_q * cos_q + _rotate_half(Q_q, jnp) * sin_q

        gb_q = jnp.tanh(gamma[0]) * (gk_q @ K_raw.T) * (D ** -0.5)

        Qh = Qr_q.reshape(Q_PER_CORE, NH, HD)
        Kh = Kr.reshape(L, NH, HD)
        Vh = V.reshape(L, NH, HD)
        logits = jnp.einsum("qhd,khd->hqk", Qh, Kh) * (HD ** -0.5)
        logits = logits + gb_q[None]
        kcol = jnp.arange(L, dtype=jnp.int32)[None, :]
        qrow = qstart + jnp.arange(Q_PER_CORE, dtype=jnp.int32)[:, None]
        logits = jnp.where((kcol <= qrow)[None], logits, jnp.float32(-1e30))
        attn = jax.nn.softmax(logits, axis=-1)
        out_q = jnp.einsum("hqk,khd->qhd", attn, Vh).reshape(Q_PER_CORE, D)
        attn_w_q = attn.mean(axis=0)
        return out_q, attn_w_q

    _PMAP_Q_CACHE["fn"] = jax.pmap(
        per_core, axis_name="i", in_axes=(0, 0, 0) + (None,) * 12)
    return _PMAP_Q_CACHE["fn"]


def _kernel_jax_q(z, pool_seq, Wq, bq, Wk, bk, Wv, bv, W_coh, gamma,
                  Wg1, bg1, Wg2, bg2):
    pmapped = _get_pmapped_q()
    z_dev = np.stack([
        z[i // GROUPS_PER_BATCH,
          (i % GROUPS_PER_BATCH) * Q_PER_CORE:(i % GROUPS_PER_BATCH + 1) * Q_PER_CORE]
        for i in range(N_CORES)])
    pool_dev = np.stack([
        pool_seq[i // GROUPS_PER_BATCH,
                 (i % GROUPS_PER_BATCH) * Q_PER_CORE:(i % GROUPS_PER_BATCH + 1) * Q_PER_CORE]
        for i in range(N_CORES)])
    qstart = np.array([(i % GROUPS_PER_BATCH) * Q_PER_CORE
                       for i in range(N_CORES)], dtype=np.int32)

    out_parts, attn_parts = pmapped(z_dev, pool_dev, qstart, Wq, bq, Wk, bk,
                                    Wv, bv, W_coh, gamma, Wg1, bg1, Wg2, bg2)
    out_parts = np.asarray(out_parts)
    attn_parts = np.asarray(attn_parts)

    out = np.empty((B, L, D), dtype=np.float32)
    attn_w = np.empty((B, L, L), dtype=np.float32)
    for i in range(N_CORES):
        b = i // GROUPS_PER_BATCH
        g = i % GROUPS_PER_BATCH
        out[b, g * Q_PER_CORE:(g + 1) * Q_PER_CORE] = out_parts[i]
        attn_w[b, g * Q_PER_CORE:(g + 1) * Q_PER_CORE] = attn_parts[i]
    return out, attn_w


def _kernel_jax(z, pool_seq, Wq, bq, Wk, bk, Wv, bv, W_coh, gamma,
                Wg1, bg1, Wg2, bg2):
    pmapped = _get_pmapped()
    z_dev = np.stack([z[i // GROUPS_PER_BATCH] for i in range(N_CORES)])
    pool_dev = np.stack([pool_seq[i // GROUPS_PER_BATCH] for i in range(N_CORES)])
    hstart = np.array([(i % GROUPS_PER_BATCH) * HEADS_PER_GROUP for i in range(N_CORES)],
                      dtype=np.int32)

    out_parts, attn_parts = pmapped(z_dev, pool_dev, hstart, Wq, bq, Wk, bk,
                                    Wv, bv, W_coh, gamma, Wg1, bg1, Wg2, bg2)
    out_parts = np.asarray(out_parts)
    attn_parts = np.asarray(attn_parts)

    out = np.empty((B, L, NH, HD), dtype=np.float32)
    attn_w = np.zeros((B, L, L), dtype=np.float32)
    for i in range(N_CORES):
        b = i // GROUPS_PER_BATCH
        g = i % GROUPS_PER_BATCH
        out[b, :, g * HEADS_PER_GROUP:(g + 1) * HEADS_PER_GROUP, :] = out_parts[i]
        attn_w[b] += attn_parts[i]
    attn_w /= NH
    return out.reshape(B, L, D), attn_w


def _kernel_numpy(z, pool_seq, Wq, bq, Wk, bk, Wv, bv, W_coh, gamma,
                  Wg1, bg1, Wg2, bg2):
    from scipy.special import erf

    cos_f, sin_f = _rope_tables(BASE_FAST)
    cos_s, sin_s = _rope_tables(BASE_SLOW)

    Q = z @ Wq.T + bq
    K_raw = z @ Wk.T + bk
    V = z @ Wv.T + bv

    gk = pool_seq @ W_coh.T
    gb = np.tanh(gamma[0]) * np.einsum("bqd,bkd->bqk", gk, K_raw) * (D ** -0.5)

    h_pre = pool_seq @ Wg1.T + bg1
    h = 0.5 * h_pre * (1.0 + erf(h_pre / np.sqrt(2.0, dtype=np.float32)))
    gate = 1.0 / (1.0 + np.exp(-(h @ Wg2.T + bg2)))

    cos = gate * cos_f + (1.0 - gate) * cos_s
    sin = gate * sin_f + (1.0 - gate) * sin_s
    Q = Q * cos + _rotate_half(Q, np) * sin
    K_rot = K_raw * cos + _rotate_half(K_raw, np) * sin

    Qh = Q.reshape(B, L, NH, HD).transpose(0, 2, 1, 3)
    Kh = K_rot.reshape(B, L, NH, HD).transpose(0, 2, 1, 3)
    Vh = V.reshape(B, L, NH, HD).transpose(0, 2, 1, 3)

    logits = np.einsum("bhqd,bhkd->bhqk", Qh, Kh).astype(np.float32) * (HD ** -0.5)
    logits = logits + gb[:, None]
    causal = np.tril(np.ones((L, L), dtype=bool))
    logits = np.where(causal[None, None], logits, np.float32(-np.inf))

    logits -= logits.max(axis=-1, keepdims=True)
    ex = np.exp(logits, dtype=np.float32)
    attn = ex / ex.sum(axis=-1, keepdims=True)
    out = np.einsum("bhqk,bhkd->bhqd", attn, Vh).astype(np.float32)
    out = out.transpose(0, 2, 1, 3).reshape(B, L, D)
    return out.astype(np.float32), attn.mean(axis=1).astype(np.float32)


def kernel(**inputs):
    inputs = {k: np.asarray(v, dtype=np.float32) for k, v in inputs.items()}
    try:
        return _kernel_jax_q(**inputs)
    except Exception:
        pass
    try:
        return _kernel_jax(**inputs)
    except Exception:
        return _kernel_numpy(**inputs)
